# revision 26
# baseline (speedup 1.0000x reference)
import numpy as np
import ml_dtypes

import concourse.bass as bass
import concourse.tile as tile
from concourse import mybir, bacc
from concourse.tile import add_dep_helper
from concourse.bass_utils import run_bass_kernel_spmd

F32 = mybir.dt.float32
BF16 = mybir.dt.bfloat16
FP8 = mybir.dt.float8e3
AF = mybir.ActivationFunctionType

B, INP, S, H = 256, 512, 256, 512
G3 = 3 * H
NC = 8
BL = B // NC
KC = H // 128
MC = G3 // 128
SQ = 64
SG = 16
LEAD = 1
WSCALE = 256.0
INV = 1.0 / WSCALE

BURST = [(0, 'n0'), (0, 'n1'), (0, 'r0'), (0, 'r1'), (0, 'z'),
         (1, 'n0'), (1, 'n1'), (1, 'r0'), (1, 'r1'), (1, 'z'),
         (2, 'r0'), (3, 'r0'), (2, 'n0'), (3, 'n0'),
         (2, 'r1'), (3, 'r1'), (2, 'n1'), (3, 'n1'), (2, 'z'), (3, 'z')]
MBASE = {'r0': 0, 'r1': 2, 'z': 4, 'n0': 8, 'n1': 10}
MCNT = {'r0': 2, 'r1': 2, 'z': 4, 'n0': 2, 'n1': 2}


def _dedup_ldweights(nc):
    removed = 0
    for f in nc.m.functions:
        for bb in f.blocks:
            insts = bb.instructions
            del_ids = set()
            last_key = None
            for i in insts:
                if type(i).__name__ == 'InstLdweights':
                    a = i.ins[0]
                    k = (a.memref, a.offset, str(a.ap), str(a.dtype),
                         str(i.perf_mode), str(i.tile_position))
                    has_sync = bool(i.sync_info and
                                    (i.sync_info.on_wait or i.sync_info.on_update))
                    if k == last_key and not has_sync:
                        del_ids.add(id(i))
                        continue
                    last_key = k
            if del_ids:
                insts[:] = [i for i in insts if id(i) not in del_ids]
            removed += len(del_ids)
    return removed


def _build(steps=S, whh_dtype=FP8):
    nc = bacc.Bacc("TRN2", target_bir_lowering=False, debug=False)

    xb_d = nc.dram_tensor("x_t", [KC, 128, steps, BL], BF16, kind="ExternalInput")
    wih_d = nc.dram_tensor("wih_t", [INP, G3], BF16, kind="ExternalInput")
    whh_d = nc.dram_tensor("whh_t", [H, G3], whh_dtype, kind="ExternalInput")
    bsum_d = nc.dram_tensor("bsum", [128, MC], F32, kind="ExternalInput")
    bhhn_d = nc.dram_tensor("bhhn", [KC, 128], BF16, kind="ExternalInput")
    sel_d = nc.dram_tensor("sel32", [KC, 128], BF16, kind="ExternalInput")
    id_d = nc.dram_tensor("idmat", [128, 128], BF16, kind="ExternalInput")
    out_d = nc.dram_tensor("h_out", [KC, 128, BL], F32, kind="ExternalOutput")

    all_mms = []

    def mm(*args, **kwargs):
        m = nc.tensor.matmul(*args, **kwargs)
        if all_mms:
            add_dep_helper(m.ins, all_mms[-1].ins, False, "pe-order")
        all_mms.append(m)
        return m

    last_dve = [None]

    def dve(instr):
        if last_dve[0] is not None:
            add_dep_helper(instr.ins, last_dve[0].ins, False, "dve-order")
        last_dve[0] = instr
        return instr

    ngroups = steps // SG

    with tile.TileContext(nc) as tc:
        with (
            tc.tile_pool(name="consts", bufs=1) as consts,
            tc.tile_pool(name="xstage", bufs=2) as xstage,
            tc.tile_pool(name="ring", bufs=3) as ringp,
            tc.tile_pool(name="ipsum", bufs=3, space="PSUM") as ipsum,
            tc.tile_pool(name="pr0", bufs=1, space="PSUM") as pr0p,
            tc.tile_pool(name="pr1", bufs=1, space="PSUM") as pr1p,
            tc.tile_pool(name="pz", bufs=1, space="PSUM") as pzp,
            tc.tile_pool(name="pn0", bufs=1, space="PSUM") as pn0p,
            tc.tile_pool(name="pn1", bufs=1, space="PSUM") as pn1p,
            tc.tile_pool(name="gates", bufs=2) as gates,
        ):
            ident = consts.tile([128, 128], BF16)
            nc.sync.dma_start(out=ident[:], in_=id_d.ap())
            wih = consts.tile([128, KC, G3], BF16)
            nc.sync.dma_start(out=wih[:],
                              in_=wih_d.rearrange("(k p) g -> p k g", p=128))
            whh = consts.tile([128, KC, G3], whh_dtype)
            bsum = consts.tile([128, MC], F32)
            bhhn = consts.tile([KC, 128], BF16)
            sel32 = consts.tile([KC, 128], BF16)

            hbf = consts.tile([128, 128], BF16)
            nc.vector.memset(hbf[:], 0.0)

            def late_consts():
                nc.sync.dma_start(
                    out=whh[:], in_=whh_d.rearrange("(k p) g -> p k g", p=128))
                nc.sync.dma_start(out=bsum[:], in_=bsum_d.ap())
                nc.sync.dma_start(out=bhhn[:], in_=bhhn_d.ap())
                nc.sync.dma_start(out=sel32[:], in_=sel_d.ap())

            slab_tiles = {}

            def stage_slab(q, k=None):
                s0 = q * SQ
                sq = min(SQ, steps - s0)
                if k is None or k == 0:
                    slab_tiles[q] = xstage.tile([128, KC, SQ, BL], BF16,
                                                name="xt", tag="xt")
                if k is None:
                    nc.sync.dma_start(
                        out=slab_tiles[q][:, :, :SG, :],
                        in_=xb_d[:, :, s0:s0 + SG, :]
                        .rearrange("k p s b -> p k s b"))
                    nc.sync.dma_start(
                        out=slab_tiles[q][:, :, SG:sq, :],
                        in_=xb_d[:, :, s0 + SG:s0 + sq, :]
                        .rearrange("k p s b -> p k s b"))
                else:
                    nc.sync.dma_start(
                        out=slab_tiles[q][:, k, :sq, :],
                        in_=xb_d[k, :, s0:s0 + sq, :])

            ring_tiles = {}
            ip_state = {}
            pending_evacs = []

            def iproj_mm(g, j):
                m_, k = j // KC, j % KC
                xt = slab_tiles[g // (SQ // SG)]
                goff = (g % (SQ // SG)) * SG
                if j == 0:
                    ring_tiles[g] = ringp.tile([128, SG, MC, BL], BF16,
                                               name="gr", tag="gr")
                if k == 0:
                    ip_state[g] = ipsum.tile([128, SG * BL], F32,
                                             name="ips", tag="ips")
                ps = ip_state[g]
                mm(ps[:], wih[:, k, 128 * m_:128 * (m_ + 1)],
                   xt[:, k, goff:goff + SG, :],
                   start=(k == 0), stop=(k == KC - 1))
                if k == KC - 1:
                    pending_evacs.append((g, m_, ps))

            def flush_evacs():
                for g, m_, ps in pending_evacs:
                    dve(nc.vector.tensor_scalar_add(
                        ring_tiles[g][:, :, m_, :],
                        ps.rearrange("p (s b) -> p s b", s=SG),
                        bsum[:, m_:m_ + 1]))
                pending_evacs.clear()

            wps = pzp.tile([128, 32], F32, name="warm", tag="pz")
            for _ in range(120):
                mm(wps[:], ident[:], ident[:, 0:32],
                   start=True, stop=True)

            stage_slab(0)
            late_consts()
            up = min(LEAD, ngroups)
            for g in range(up):
                for m_ in range(MC):
                    for k in range(KC):
                        iproj_mm(g, m_ * KC + k)
                    flush_evacs()

            def make_pg(t):
                gr = ring_tiles[t // SG]
                tcol = t % SG
                p_g = {'r0': pr0p.tile([128, 64], F32, name="pr0", tag="pr0"),
                       'r1': pr1p.tile([128, 64], F32, name="pr1", tag="pr1"),
                       'z': pzp.tile([128, 128], F32, name="pz", tag="pz"),
                       'n0': pn0p.tile([128, 64], F32, name="pn0", tag="pn0"),
                       'n1': pn1p.tile([128, 64], F32, name="pn1", tag="pn1")}
                mm(p_g['r0'][:], ident[:],
                   gr[:, tcol, 0:2, :].rearrange("p m b -> p (m b)"),
                   start=True, stop=False)
                mm(p_g['n0'][:], bhhn[:], sel32[:, 0:64],
                   start=True, stop=False)
                mm(p_g['n1'][:], bhhn[:], sel32[:, 64:128],
                   start=True, stop=False)
                mm(p_g['z'][:], ident[:],
                   gr[:, tcol, 4:8, :].rearrange("p m b -> p (m b)"),
                   start=True, stop=False)
                mm(p_g['r1'][:], ident[:],
                   gr[:, tcol, 2:4, :].rearrange("p m b -> p (m b)"),
                   start=True, stop=False)
                return p_g

            pend_pg = [None]

            for t in range(steps):
                for q in range(1, (steps + SQ - 1) // SQ):
                    for kk in range(KC):
                        if t == max(0, SQ * q - 58) + 2 * kk:
                            stage_slab(q, kk)

                gr = ring_tiles[t // SG]
                tcol = t % SG

                if pend_pg[0] is not None:
                    p_g = pend_pg[0]
                    pend_pg[0] = None
                else:
                    p_g = make_pg(t)

                for k, blk in BURST:
                    for mi in range(MCNT[blk]):
                        m_ = MBASE[blk] + mi
                        mm(p_g[blk][:, 32 * mi:32 * (mi + 1)],
                           whh[:, k, 128 * m_:128 * (m_ + 1)],
                           hbf[:, 32 * k:32 * (k + 1)],
                           start=False,
                           stop=(k == KC - 1) and (mi == MCNT[blk] - 1))

                g = t // SG + LEAD
                if g < ngroups:
                    j0 = 3 * (t % SG)
                    for j in (j0, j0 + 1, j0 + 2):
                        iproj_mm(g, j)
                if t + 1 < steps:
                    pend_pg[0] = make_pg(t + 1)

                def T(nm, cols=64):
                    return gates.tile([128, cols], BF16, name=nm, tag=nm)

                def gn_ap(h_):
                    return (gr[:, tcol, 8 + 2 * h_:10 + 2 * h_, :]
                            .rearrange("p m b -> p (m b)"))

                rt = {0: T("rt0"), 1: T("rt1")}
                zt = T("zt", 128)
                nc.scalar.activation(rt[0][:], p_g['r0'][:], AF.Sigmoid,
                                     scale=INV)
                nc.scalar.activation(rt[1][:], p_g['r1'][:], AF.Sigmoid,
                                     scale=INV)
                nc.scalar.activation(zt[:], p_g['z'][:], AF.Sigmoid, scale=INV)
                u, tt, vv, nn, t1 = {}, {}, {}, {}, {}
                for h_ in range(2):
                    u[h_], tt[h_], vv[h_] = T(f"u{h_}"), T(f"tt{h_}"), T(f"vv{h_}")
                    nn[h_], t1[h_] = T(f"nn{h_}"), T(f"t1{h_}")
                nc.gpsimd.tensor_mul(u[0][:], zt[:, 0:64], hbf[:, 0:64])
                nc.gpsimd.tensor_mul(u[1][:], zt[:, 64:128], hbf[:, 64:128])
                for h_ in range(2):
                    dve(nc.vector.tensor_mul(tt[h_][:], rt[h_][:],
                                             p_g['n%d' % h_][:]))
                    dve(nc.vector.tensor_add(vv[h_][:], tt[h_][:], gn_ap(h_)))
                nc.scalar.activation(nn[0][:], vv[0][:], AF.Tanh, scale=INV)
                nc.scalar.activation(nn[1][:], vv[1][:], AF.Tanh, scale=INV)
                for h_ in range(2):
                    c = slice(64 * h_, 64 * (h_ + 1))
                    dve(nc.vector.scalar_tensor_tensor(
                        t1[h_][:], zt[:, c], -1.0, nn[h_][:],
                        mybir.AluOpType.add, mybir.AluOpType.mult))
                    dve(nc.vector.tensor_sub(hbf[:, c], u[h_][:], t1[h_][:]))

                flush_evacs()

            h32 = gates.tile([128, 128], F32, name="hout", tag="hout")
            nc.vector.tensor_copy(h32[:], hbf[:])
            for k in range(KC):
                nc.sync.dma_start(out=out_d[k], in_=h32[:, 32 * k:32 * (k + 1)])

    nc.compile()
    _dedup_ldweights(nc)
    return nc


def _prep_inputs(x, weight_ih, weight_hh, bias_ih, bias_hh,
                 whh_np=ml_dtypes.float8_e3m4):
    x = np.ascontiguousarray(np.asarray(x, dtype=np.float32))
    w_ih = np.asarray(weight_ih, dtype=np.float32)
    w_hh = np.asarray(weight_hh, dtype=np.float32)
    b_ih = np.asarray(bias_ih, dtype=np.float32)
    b_hh = np.asarray(bias_hh, dtype=np.float32)

    wih_t = np.ascontiguousarray(w_ih.T * WSCALE).astype(ml_dtypes.bfloat16)
    whh_t = np.ascontiguousarray(w_hh.T * WSCALE).astype(whh_np)
    bsum = np.empty((128, MC), np.float32)
    for m in range(MC):
        seg = b_ih[128 * m:128 * (m + 1)].copy()
        if m < 8:
            seg += b_hh[128 * m:128 * (m + 1)]
        bsum[:, m] = seg * WSCALE
    bhhn = (b_hh[2 * H:] * WSCALE).reshape(KC, 128).astype(ml_dtypes.bfloat16)
    sel32 = np.zeros((KC, 128), np.float32)
    for k in range(KC):
        sel32[k, 32 * k:32 * (k + 1)] = 1.0
    sel32 = sel32.astype(ml_dtypes.bfloat16)
    idmat = np.eye(128, dtype=np.float32).astype(ml_dtypes.bfloat16)

    x_bf = x.astype(ml_dtypes.bfloat16)

    shared = {"wih_t": wih_t, "whh_t": whh_t, "bsum": bsum,
              "bhhn": bhhn, "sel32": sel32, "idmat": idmat}
    in_maps = []
    for c in range(NC):
        m = dict(shared)
        xc = x_bf[BL * c:BL * (c + 1)]
        xt = xc.transpose(1, 2, 0).reshape(KC, 128, S, BL)
        m["x_t"] = np.ascontiguousarray(xt)
        in_maps.append(m)
    return in_maps


_NC_CACHE = {}


def _get_nc(steps=S):
    if steps not in _NC_CACHE:
        _NC_CACHE[steps] = _build(steps)
    return _NC_CACHE[steps]


def kernel(x, weight_ih, weight_hh, bias_ih, bias_hh):
    nc = _get_nc(S)
    in_maps = _prep_inputs(x, weight_ih, weight_hh, bias_ih, bias_hh)
    res = run_bass_kernel_spmd(nc, in_maps, core_ids=list(range(NC)))
    outs = []
    for c in range(NC):
        hT = np.asarray(res.results[c]["h_out"])
        outs.append(np.transpose(hT, (2, 0, 1)).reshape(BL, H))
    return np.concatenate(outs, axis=0).astype(np.float32)
